# revision 10
# baseline (speedup 1.0000x reference)
"""VQ codebook kernel for Trainium2 (8 NeuronCores, data-parallel over batch).

Computes, for x: [32, 64, 4096] f32 and weight: [1024, 64] f32:
  indices[b, t]  = argmin_k || x[b, :, t] - weight[k] ||^2
  quantized      = transpose(weight[indices], (0, 2, 1))   # [32, 64, 4096]

Device algorithm (per core, 4 batches of x):
  - scores s[t, k] = 2*x_t.w_k - ||w_k||^2 via PE matmuls (argmax s == argmin
    dist).  fp32 64-contraction matmuls are row-packed: token tiles 2i / 2i+1
    run concurrently on PE row strips 0-63 / 64-127 (x and the codebook are
    duplicated across both partition halves).  The -||w||^2 bias accumulates
    via a 2-row fp16 matmul (bias split hi+lo, exact to ~4e-6).
  - exact fp32 argmax per token: DVE max8 for the max value, then DVE
    max_index (FIND_INDEX8) for the first-occurrence index.
  - codebook row gather via indirect DMA, PE-transpose to [D, T], DMA out.
"""

import sys

if "/opt/trn_rl_repo" not in sys.path:
    sys.path.insert(0, "/opt/trn_rl_repo")

import numpy as np

B, D, T = 32, 64, 4096
K = 1024
NCORES = 8
B_LOC = B // NCORES          # batches per core
TILE_T = 128                 # tokens per tile (partition dim)
NT = T // TILE_T             # token tiles per batch

_CACHE = {}


def _build_program():
    import concourse.mybir as mybir
    from concourse import bacc
    import concourse.bass as bass
    from concourse.tile import TileContext
    from concourse.masks import make_identity

    fp32 = mybir.dt.float32
    fp16 = mybir.dt.float16
    u32 = mybir.dt.uint32

    nc = bacc.Bacc("TRN2")
    # x duplicated into both partition halves: rows 0-63 and 64-127 both = x[b]
    xa = nc.dram_tensor("xa", [B_LOC, 2 * D, T], fp32, kind="ExternalInput")
    # wp = 2*w.T duplicated into both halves: [128, K]
    wp = nc.dram_tensor("wp", [2 * D, K], fp32, kind="ExternalInput")
    # bias rows (fp16): rows 0,1 = hi,lo of -||w||^2 ; rows 2,3 = same (dup for
    # strip B at partitions 64,65)
    bb = nc.dram_tensor("bb", [4, K], fp16, kind="ExternalInput")
    wg = nc.dram_tensor("wg", [K, D], fp32, kind="ExternalInput")
    q = nc.dram_tensor("q", [B_LOC, D, T], fp32, kind="ExternalOutput")
    idxo = nc.dram_tensor("idxo", [B_LOC, TILE_T, NT], u32, kind="ExternalOutput")

    with TileContext(nc) as tc:
        with (
            tc.tile_pool(name="const", bufs=1) as constp,
            tc.tile_pool(name="xbuf", bufs=2) as xbufp,
            tc.tile_pool(name="work", bufs=8) as workp,
            tc.tile_pool(name="outb", bufs=2) as outbp,
            tc.tile_pool(name="psum", bufs=3, space="PSUM") as psump,
            tc.tile_pool(name="psumt", bufs=2, space="PSUM") as psumtp,
        ):
            wp_sb = constp.tile([2 * D, K], fp32)
            nc.sync.dma_start(wp_sb[:], wp[:])
            # bias tensor: [128, K] fp16, rows 0-1 and 64-65 hold hi/lo rows
            bb_sb = constp.tile([128, K], fp16)
            nc.sync.dma_start(bb_sb[0:2, :], bb[0:2, :])
            nc.sync.dma_start(bb_sb[64:66, :], bb[2:4, :])
            ones_sb = constp.tile([128, TILE_T], fp16)
            nc.vector.memset(ones_sb[:], 1.0)
            ident = constp.tile([128, 128], fp32)
            make_identity(nc, ident[:])

            for b in range(B_LOC):
                xa_sb = xbufp.tile([2 * D, T], fp32, tag="xa")
                nc.sync.dma_start(xa_sb[:], xa[b])
                idx8 = outbp.tile([TILE_T, NT, 8], u32, tag="idx8")
                qt = outbp.tile([D, T], fp32, tag="qt")
                for tp in range(NT // 2):
                    t0, t1 = 2 * tp, 2 * tp + 1
                    psA = psump.tile([TILE_T, K], fp32, tag="ps")
                    psB = psump.tile([TILE_T, K], fp32, tag="ps")
                    lhsA = xa_sb[0:D, t0 * TILE_T : (t0 + 1) * TILE_T]
                    lhsB = xa_sb[D : 2 * D, t1 * TILE_T : (t1 + 1) * TILE_T]
                    # f32 score matmuls, row strips A (0-63) and B (64-127)
                    nc.tensor.matmul(
                        psA[:, 0:512], lhsA, wp_sb[0:D, 0:512],
                        start=True, stop=False, tile_position=(0, 0),
                        skip_group_check=True,
                    )
                    nc.tensor.matmul(
                        psB[:, 0:512], lhsB, wp_sb[D : 2 * D, 0:512],
                        start=True, stop=False, tile_position=(64, 0),
                        skip_group_check=True,
                    )
                    nc.tensor.matmul(
                        psA[:, 512:1024], lhsA, wp_sb[0:D, 512:1024],
                        start=True, stop=False, tile_position=(0, 0),
                        skip_group_check=True,
                    )
                    nc.tensor.matmul(
                        psB[:, 512:1024], lhsB, wp_sb[D : 2 * D, 512:1024],
                        start=True, stop=False, tile_position=(64, 0),
                        skip_group_check=True,
                    )
                    # bias accumulate: 512-col fp16 matmuls (one PSUM bank each)
                    for h0, h1 in ((0, 512), (512, 1024)):
                        nc.tensor.matmul(
                            psA[:, h0:h1], ones_sb[0:2, :], bb_sb[0:2, h0:h1],
                            start=False, stop=True, tile_position=(0, 0),
                            skip_group_check=True,
                        )
                        nc.tensor.matmul(
                            psB[:, h0:h1], ones_sb[64:66, :], bb_sb[64:66, h0:h1],
                            start=False, stop=True, tile_position=(64, 0),
                            skip_group_check=True,
                        )
                    for ps, tt in ((psA, t0), (psB, t1)):
                        sbs = workp.tile([TILE_T, K], fp32, tag="sbs")
                        nc.scalar.copy(sbs[:], ps[:])
                        m8 = workp.tile([TILE_T, 8], fp32, tag="m8")
                        nc.vector.max(m8[:], sbs[:])
                        nc.vector.max_index(
                            out=idx8[:, tt, :],
                            in_max=m8[:, 0:1].to_broadcast([TILE_T, 8]),
                            in_values=sbs[:],
                        )
                        gq = workp.tile([TILE_T, D], fp32, tag="gq")
                        nc.gpsimd.indirect_dma_start(
                            out=gq[:],
                            out_offset=None,
                            in_=wg[:],
                            in_offset=bass.IndirectOffsetOnAxis(
                                ap=idx8[:, tt, 0:1], axis=0
                            ),
                        )
                        pst = psumtp.tile([D, TILE_T], fp32, tag="pst")
                        nc.tensor.transpose(pst[:], gq[:], ident[:])
                        nc.scalar.copy(qt[:, tt * TILE_T : (tt + 1) * TILE_T], pst[:])
                nc.sync.dma_start(q[b], qt[:])
                nc.sync.dma_start(idxo[b], idx8[:, :, 0])
    nc.compile()
    return nc


def _get_program():
    if "nc" not in _CACHE:
        _CACHE["nc"] = _build_program()
    return _CACHE["nc"]


def _split_hi_lo_f16(v):
    hi = v.astype(np.float16)
    lo = (v - hi.astype(np.float32)).astype(np.float16)
    return hi, lo


def kernel(x, weight):
    from concourse.bass_utils import run_bass_kernel_spmd

    x = np.asarray(x, dtype=np.float32)
    weight = np.asarray(weight, dtype=np.float32)

    w2t = 2.0 * weight.T                                  # [D, K]
    wp = np.concatenate([w2t, w2t], axis=0)               # [128, K]
    nb = -np.sum(weight * weight, axis=1, dtype=np.float32)  # [K]
    bh, bl = _split_hi_lo_f16(nb)
    bb = np.stack([bh, bl, bh, bl], axis=0)               # [4, K] fp16

    in_maps = []
    for c in range(NCORES):
        xs = x[c * B_LOC : (c + 1) * B_LOC]               # [B_LOC, D, T]
        xa = np.concatenate([xs, xs], axis=1)             # [B_LOC, 128, T]
        in_maps.append(
            {
                "xa": np.ascontiguousarray(xa),
                "wp": np.ascontiguousarray(wp),
                "bb": np.ascontiguousarray(bb),
                "wg": weight,
            }
        )

    nc = _get_program()
    import os
    trace = bool(os.environ.get("BASS_TRACE"))
    res = run_bass_kernel_spmd(nc, in_maps, core_ids=list(range(NCORES)), trace=trace)
    if res.exec_time_ns is not None:
        _CACHE["exec_time_ns"] = res.exec_time_ns

    quantized = np.empty((B, D, T), dtype=np.float32)
    indices = np.empty((B, T), dtype=np.int32)
    for c in range(NCORES):
        r = res.results[c]
        quantized[c * B_LOC : (c + 1) * B_LOC] = r["q"]
        # idxo [B_LOC, 128, NT]: token t = tt*128 + p  ->  [B_LOC, NT, 128]
        idx = np.transpose(r["idxo"], (0, 2, 1)).reshape(B_LOC, T)
        indices[c * B_LOC : (c + 1) * B_LOC] = idx.astype(np.int32)
    return quantized, indices


# revision 11
# speedup vs baseline: 1.0249x; 1.0249x over previous
"""VQ codebook kernel for Trainium2 (8 NeuronCores, data-parallel over batch).

Computes, for x: [32, 64, 4096] f32 and weight: [1024, 64] f32:
  indices[b, t]  = argmin_k || x[b, :, t] - weight[k] ||^2
  quantized      = transpose(weight[indices], (0, 2, 1))   # [32, 64, 4096]

Device algorithm (per core, 4 batches of x):
  - scores s[t, k] = 2*x_t.w_k - ||w_k||^2 via PE matmuls (argmax s == argmin
    dist).  fp32 64-contraction matmuls are row-packed: token tiles 2i / 2i+1
    run concurrently on PE row strips 0-63 / 64-127 (x and the codebook are
    duplicated across both partition halves).  The -||w||^2 bias accumulates
    via a 2-row fp16 matmul (bias split hi+lo, exact to ~4e-6).
  - exact fp32 argmax per token: DVE max8 for the max value, then DVE
    max_index (FIND_INDEX8) for the first-occurrence index.
  - codebook row gather via indirect DMA, PE-transpose to [D, T], DMA out.
"""

import sys

if "/opt/trn_rl_repo" not in sys.path:
    sys.path.insert(0, "/opt/trn_rl_repo")

import numpy as np

B, D, T = 32, 64, 4096
K = 1024
NCORES = 8
B_LOC = B // NCORES          # batches per core
TILE_T = 128                 # tokens per tile (partition dim)
NT = T // TILE_T             # token tiles per batch

_CACHE = {}


def _build_program():
    import concourse.mybir as mybir
    from concourse import bacc
    import concourse.bass as bass
    from concourse.tile import TileContext
    from concourse.masks import make_identity

    fp32 = mybir.dt.float32
    fp16 = mybir.dt.float16
    u32 = mybir.dt.uint32

    nc = bacc.Bacc("TRN2")
    # x duplicated into both partition halves: rows 0-63 and 64-127 both = x[b]
    xa = nc.dram_tensor("xa", [B_LOC, 2 * D, T], fp32, kind="ExternalInput")
    # wp = 2*w.T duplicated into both halves: [128, K]
    wp = nc.dram_tensor("wp", [2 * D, K], fp32, kind="ExternalInput")
    # bias rows (fp16): rows 0,1 = hi,lo of -||w||^2 ; rows 2,3 = same (dup for
    # strip B at partitions 64,65)
    bb = nc.dram_tensor("bb", [4, K], fp16, kind="ExternalInput")
    wg = nc.dram_tensor("wg", [K, D], fp32, kind="ExternalInput")
    q = nc.dram_tensor("q", [B_LOC, D, T], fp32, kind="ExternalOutput")
    idxo = nc.dram_tensor("idxo", [B_LOC, TILE_T, NT], u32, kind="ExternalOutput")

    with TileContext(nc) as tc:
        with (
            tc.tile_pool(name="const", bufs=1) as constp,
            tc.tile_pool(name="xbuf", bufs=3) as xbufp,
            tc.tile_pool(name="work", bufs=8) as workp,
            tc.tile_pool(name="outb", bufs=3) as outbp,
            tc.tile_pool(name="psum", bufs=3, space="PSUM") as psump,
            tc.tile_pool(name="psumt", bufs=2, space="PSUM") as psumtp,
        ):
            wp_sb = constp.tile([2 * D, K], fp32)
            nc.sync.dma_start(wp_sb[:], wp[:])
            # bias tensor: [128, K] fp16, rows 0-1 and 64-65 hold hi/lo rows
            bb_sb = constp.tile([128, K], fp16)
            nc.sync.dma_start(bb_sb[0:2, :], bb[0:2, :])
            nc.sync.dma_start(bb_sb[64:66, :], bb[2:4, :])
            ones_sb = constp.tile([128, TILE_T], fp16)
            nc.vector.memset(ones_sb[:], 1.0)
            ident = constp.tile([128, 128], fp32)
            make_identity(nc, ident[:])

            for b in range(B_LOC):
                xa_sb = xbufp.tile([2 * D, T], fp32, tag="xa")
                nc.sync.dma_start(xa_sb[:], xa[b])
                idx8 = outbp.tile([TILE_T, NT, 8], u32, tag="idx8")
                qt = outbp.tile([D, T], fp32, tag="qt")
                for tp in range(NT // 2):
                    t0, t1 = 2 * tp, 2 * tp + 1
                    psA = psump.tile([TILE_T, K], fp32, tag="ps")
                    psB = psump.tile([TILE_T, K], fp32, tag="ps")
                    lhsA = xa_sb[0:D, t0 * TILE_T : (t0 + 1) * TILE_T]
                    lhsB = xa_sb[D : 2 * D, t1 * TILE_T : (t1 + 1) * TILE_T]
                    # f32 score matmuls, row strips A (0-63) and B (64-127)
                    nc.tensor.matmul(
                        psA[:, 0:512], lhsA, wp_sb[0:D, 0:512],
                        start=True, stop=False, tile_position=(0, 0),
                        skip_group_check=True,
                    )
                    nc.tensor.matmul(
                        psB[:, 0:512], lhsB, wp_sb[D : 2 * D, 0:512],
                        start=True, stop=False, tile_position=(64, 0),
                        skip_group_check=True,
                    )
                    nc.tensor.matmul(
                        psA[:, 512:1024], lhsA, wp_sb[0:D, 512:1024],
                        start=True, stop=False, tile_position=(0, 0),
                        skip_group_check=True,
                    )
                    nc.tensor.matmul(
                        psB[:, 512:1024], lhsB, wp_sb[D : 2 * D, 512:1024],
                        start=True, stop=False, tile_position=(64, 0),
                        skip_group_check=True,
                    )
                    # bias accumulate: 512-col fp16 matmuls (one PSUM bank each)
                    for h0, h1 in ((0, 512), (512, 1024)):
                        nc.tensor.matmul(
                            psA[:, h0:h1], ones_sb[0:2, :], bb_sb[0:2, h0:h1],
                            start=False, stop=True, tile_position=(0, 0),
                            skip_group_check=True,
                        )
                        nc.tensor.matmul(
                            psB[:, h0:h1], ones_sb[64:66, :], bb_sb[64:66, h0:h1],
                            start=False, stop=True, tile_position=(64, 0),
                            skip_group_check=True,
                        )
                    for ps, tt in ((psA, t0), (psB, t1)):
                        sbs = workp.tile([TILE_T, K], fp32, tag="sbs")
                        nc.scalar.copy(sbs[:], ps[:])
                        m8 = workp.tile([TILE_T, 8], fp32, tag="m8")
                        nc.vector.max(m8[:], sbs[:])
                        nc.vector.max_index(
                            out=idx8[:, tt, :],
                            in_max=m8[:, 0:1].to_broadcast([TILE_T, 8]),
                            in_values=sbs[:],
                        )
                        gq = workp.tile([TILE_T, D], fp32, tag="gq")
                        nc.gpsimd.indirect_dma_start(
                            out=gq[:],
                            out_offset=None,
                            in_=wg[:],
                            in_offset=bass.IndirectOffsetOnAxis(
                                ap=idx8[:, tt, 0:1], axis=0
                            ),
                        )
                        pst = psumtp.tile([D, TILE_T], fp32, tag="pst")
                        nc.tensor.transpose(pst[:], gq[:], ident[:])
                        nc.scalar.copy(qt[:, tt * TILE_T : (tt + 1) * TILE_T], pst[:])
                nc.sync.dma_start(q[b], qt[:])
                nc.sync.dma_start(idxo[b], idx8[:, :, 0])
    nc.compile()
    return nc


def _get_program():
    if "nc" not in _CACHE:
        _CACHE["nc"] = _build_program()
    return _CACHE["nc"]


def _split_hi_lo_f16(v):
    hi = v.astype(np.float16)
    lo = (v - hi.astype(np.float32)).astype(np.float16)
    return hi, lo


def kernel(x, weight):
    from concourse.bass_utils import run_bass_kernel_spmd

    x = np.asarray(x, dtype=np.float32)
    weight = np.asarray(weight, dtype=np.float32)

    w2t = 2.0 * weight.T                                  # [D, K]
    wp = np.concatenate([w2t, w2t], axis=0)               # [128, K]
    nb = -np.sum(weight * weight, axis=1, dtype=np.float32)  # [K]
    bh, bl = _split_hi_lo_f16(nb)
    bb = np.stack([bh, bl, bh, bl], axis=0)               # [4, K] fp16

    in_maps = []
    for c in range(NCORES):
        xs = x[c * B_LOC : (c + 1) * B_LOC]               # [B_LOC, D, T]
        xa = np.concatenate([xs, xs], axis=1)             # [B_LOC, 128, T]
        in_maps.append(
            {
                "xa": np.ascontiguousarray(xa),
                "wp": np.ascontiguousarray(wp),
                "bb": np.ascontiguousarray(bb),
                "wg": weight,
            }
        )

    nc = _get_program()
    import os
    trace = bool(os.environ.get("BASS_TRACE"))
    res = run_bass_kernel_spmd(nc, in_maps, core_ids=list(range(NCORES)), trace=trace)
    if res.exec_time_ns is not None:
        _CACHE["exec_time_ns"] = res.exec_time_ns

    quantized = np.empty((B, D, T), dtype=np.float32)
    indices = np.empty((B, T), dtype=np.int32)
    for c in range(NCORES):
        r = res.results[c]
        quantized[c * B_LOC : (c + 1) * B_LOC] = r["q"]
        # idxo [B_LOC, 128, NT]: token t = tt*128 + p  ->  [B_LOC, NT, 128]
        idx = np.transpose(r["idxo"], (0, 2, 1)).reshape(B_LOC, T)
        indices[c * B_LOC : (c + 1) * B_LOC] = idx.astype(np.int32)
    return quantized, indices
